# revision 1
# baseline (speedup 1.0000x reference)
"""Causal self-attention (B=4, L=2048, D=1024, H=16) on 8 Trainium2 NeuronCores.

Sharding: core c handles batch b = c//2 and head-group hg = c%2 (8 of 16 heads).
Each core computes its local QKV projection, causal flash-style attention for
its 8 heads, and a partial output projection against its 512 columns of
proj_w. The host sums the two partial outputs per batch and adds proj_b.

Device layouts (per core):
  xT    [1024, L]   x[b].T              (contraction dim d on partitions)
  w_qk  [1024,1024] qkv_w local q+k rows, transposed; q part pre-scaled by
                    HEAD_DIM**-0.5 (folded into weights+bias)
  w_v   [1024, 512] qkv_w local v rows, transposed
  qkT   [1024, L]   (q;k) features on partitions, tokens on free dim
  vaug  [L, 8*65]   v in token-major layout, one extra ones-column per head
                    (the ones column makes the PV matmul also produce the
                    softmax denominator as psum row 64)
  yT    [512, L]    normalized attention output, features on partitions
  projT [512, 1024] proj_w local columns, transposed

Softmax skips the max-subtraction (scores are O(+-10) for this problem's
scale, far from fp32 overflow) so P = exp(S); masking is a multiplicative
0/1 mask applied only on blocks that the host classifies as partial. Blocks
that are fully masked are skipped; fully-kept blocks skip the mask multiply.
"""

import os
import sys

import numpy as np

for _p in ("/opt/trn_rl_repo", "/root/.axon_site/_ro/trn_rl_repo"):
    if os.path.isdir(_p) and _p not in sys.path:
        sys.path.append(_p)

import ml_dtypes  # noqa: E402
import concourse.bass as bass  # noqa: E402
import concourse.tile as tile  # noqa: E402
from concourse import bacc, mybir  # noqa: E402
from concourse.bass_utils import run_bass_kernel_spmd  # noqa: E402

DIM = 1024
NUM_HEADS = 16
HEAD_DIM = 64
SCALE = HEAD_DIM**-0.5
B = 4
L = 2048
NCORES = 8
HLOC = 8  # heads per core

F32 = mybir.dt.float32
BF16 = mybir.dt.bfloat16
F32R = mybir.dt.float32r

# dtype config per matmul stage: "f32" | "bf16" | "f32r"
# f32r stores fp32 bits and bitcasts the matmul operands to float32r
# (full-rate PE streaming, ~tf32 multiply precision).
DEFAULT_CFG = (
    os.environ.get("MM_IN", "bf16"),    # x / w_qk / w_v (QKV projection)
    os.environ.get("MM_QK", "bf16"),    # q/k storage + scores matmul
    os.environ.get("MM_PV", "bf16"),    # P / v_aug (attention-weighted sum)
    os.environ.get("MM_PROJ", "bf16"),  # yT / projT (output projection)
    os.environ.get("MM_REP", "f32"),    # (unused since gpsimd broadcast)
)


def _st(c):
    """Storage dtype for a config string."""
    return {"bf16": BF16, "f32r": F32R, "f32": F32}[c]


def _np_st(c):
    return ml_dtypes.bfloat16 if c == "bf16" else np.float32


def _mm(ap, c):
    """Matmul-operand view of an AP for a config string."""
    return ap


def schedule_from_mask(am, Lc):
    """Classify [128k x 512q] score blocks from attn_mask[q, k].

    Returns (sched, patterns): sched[qj] = list of (ki, pat_idx|None) blocks
    to compute; patterns = list of [128, 512] float32 0/1 arrays (k on
    partitions, q on free dim) for partially-masked blocks.
    """
    am = np.asarray(am) != 0
    sched, patterns, pat_ids = [], [], {}
    for qj in range(Lc // 512):
        row = []
        for ki in range(Lc // 128):
            blk = am[qj * 512:(qj + 1) * 512, ki * 128:(ki + 1) * 128]  # [q,k]
            if not blk.any():
                continue
            if blk.all():
                row.append((ki, None))
                continue
            pat = np.ascontiguousarray(blk.T).astype(np.float32)  # [k,q]
            key = pat.tobytes()
            if key not in pat_ids:
                pat_ids[key] = len(patterns)
                patterns.append(pat)
            row.append((ki, pat_ids[key]))
        sched.append(row)
    return sched, patterns


def build_nc(Lc, sched, n_pat, cfg, nrep=1, phases=3):
    """Emit the per-core Bass/Tile program. Same program runs on all cores.

    nrep > 1 wraps the whole body in an on-device For_i loop — used only
    by the dev timing harness to amortize host/dispatch overhead.
    phases: 1 = QKV only, 2 = +attention, 3 = full (dev decomposition).
    """
    c_in, c_qk, c_pv, c_proj, c_rep = cfg
    dt_in, dt_qk, dt_pv, dt_proj, dt_rep = (
        _st(c_in), _st(c_qk), _st(c_pv), _st(c_proj), _st(c_rep))

    NLB = Lc // 512   # l-blocks (also q-blocks)
    NKT = Lc // 128   # k-tiles
    ND = DIM // 128   # contraction tiles for QKV
    assert n_pat <= 16, f"too many distinct mask patterns ({n_pat})"

    nc = bacc.Bacc("TRN2", target_bir_lowering=False, debug=False)

    dt_c = F32 if dt_in == F32R else dt_in
    dt_rc = F32 if dt_rep == F32R else dt_rep

    xT = nc.dram_tensor("xT", [DIM, Lc], dt_in, kind="ExternalInput")
    w_qk = nc.dram_tensor("w_qk", [DIM, 1024], dt_in, kind="ExternalInput")
    w_v = nc.dram_tensor("w_v", [DIM, 512], dt_in, kind="ExternalInput")
    bqk = nc.dram_tensor("bqk", [128, 8], F32, kind="ExternalInput")
    bv = nc.dram_tensor("bv", [1, 512], dt_c, kind="ExternalInput")
    masks = nc.dram_tensor(
        "masks", [max(n_pat, 1), 128, 512], dt_pv, kind="ExternalInput")
    projT = nc.dram_tensor("projT", [512, 1024], dt_proj, kind="ExternalInput")
    y = nc.dram_tensor("y", [Lc, 1024], F32, kind="ExternalOutput")

    with tile.TileContext(nc) as tc:
        import contextlib
        with contextlib.ExitStack() as ctx:
            sing = ctx.enter_context(tc.tile_pool(name="sing", bufs=1))

            # persistent buffers
            qkT = [sing.tile([128, Lc], dt_qk, tag=f"qkT{t}", name=f"qkT{t}")
                   for t in range(8)]
            vaug = [sing.tile([128, HLOC * 65], dt_pv, tag=f"vaug{t}",
                              name=f"vaug{t}") for t in range(NKT)]
            yT = [sing.tile([128, Lc], dt_proj, tag=f"yT{t}", name=f"yT{t}")
                  for t in range(4)]
            projT_sb = [sing.tile([128, 1024], dt_proj, tag=f"pw{t}",
                                  name=f"pw{t}") for t in range(4)]
            wv_sb = [sing.tile([128, 512], dt_in, tag=f"wv{t}", name=f"wv{t}")
                     for t in range(ND)]
            # constant/bias tiles stay plain f32 — memset can't emit f32r,
            # and the rank-1 matmuls that read them don't need f32r speed
            bqk_sb = sing.tile([128, 8], F32, tag="bqk_sb", name="bqk_sb")
            bv_sb = sing.tile([1, 512], dt_c, tag="bv_sb", name="bv_sb")
            mask_sb = [sing.tile([128, 512], dt_pv, tag=f"msk{p}",
                                 name=f"msk{p}") for p in range(n_pat)]
            ones_col = sing.tile([1, 128], dt_c, tag="ones_col",
                                 name="ones_col")

            nc.vector.memset(ones_col[:, :], 1.0)
            for t in range(NKT):
                va = vaug[t].rearrange("p (h c) -> p h c", c=65)
                nc.vector.memset(va[:, :, 64:65], 1.0)

            for t in range(ND):
                nc.sync.dma_start(wv_sb[t][:, :], w_v[t * 128:(t + 1) * 128, :])
            for t in range(4):
                nc.sync.dma_start(projT_sb[t][:, :],
                                  projT[t * 128:(t + 1) * 128, :])
            nc.sync.dma_start(bqk_sb[:, :], bqk[:, :])
            nc.sync.dma_start(bv_sb[:, :], bv[:, :])
            for p in range(n_pat):
                nc.sync.dma_start(mask_sb[p][:, :], masks[p, :, :])

            # streaming pools
            xp = ctx.enter_context(tc.tile_pool(name="xp", bufs=ND + 2))
            xw = ctx.enter_context(tc.tile_pool(name="xw", bufs=ND + 4))
            ptp = ctx.enter_context(tc.tile_pool(name="ptp", bufs=4))
            osp = ctx.enter_context(tc.tile_pool(name="osp", bufs=3))
            rsp = ctx.enter_context(tc.tile_pool(name="rsp", bufs=4))
            repp = ctx.enter_context(tc.tile_pool(name="repp", bufs=3))
            outp = ctx.enter_context(tc.tile_pool(name="outp", bufs=3))
            # one shared psum pool: qkv/proj [128,512] and score [128,1024]
            # tiles share the same tag, so the 3 two-bank slots serve
            # whichever phase is active (6 banks) + 2 banks for pov = 8.
            # (measured: splitting pools 2/2/2 or rebalancing 2/4 is WORSE —
            # attention needs 3 score slots to keep the ACT exp pipe full)
            pmm = ctx.enter_context(tc.tile_pool(name="pmm", bufs=3,
                                                 space="PSUM"))
            psc = pmm
            pov = ctx.enter_context(tc.tile_pool(name="pov", bufs=2,
                                                 space="PSUM"))

            rep_ctx = (tc.For_i(0, nrep, 1) if nrep > 1
                       else contextlib.nullcontext())
            with rep_ctx:
                for lb in range(NLB):
                    l0 = lb * 512
                    # ---- QKV projection for this l-block -------------------
                    xt = []
                    for d in range(ND):
                        xd = xp.tile([128, 512], dt_in, tag="xd", name=f"x{lb}_{d}")
                        nc.sync.dma_start(
                            xd[:, :], xT[d * 128:(d + 1) * 128, l0:l0 + 512])
                        xt.append(xd)
                    for eh in range(2):
                        wt = []
                        for d in range(ND):
                            wd = xw.tile([128, 512], dt_in, tag="wd",
                                         name=f"w{lb}_{eh}_{d}")
                            nc.sync.dma_start(
                                wd[:, :],
                                w_qk[d * 128:(d + 1) * 128,
                                     eh * 512:(eh + 1) * 512])
                            wt.append(wd)
                        for e4 in range(4):
                            e = eh * 4 + e4
                            ps = pmm.tile([128, 512], F32, tag="ps",
                                          name=f"q{lb}_{e}")
                            for d in range(ND):
                                nc.tensor.matmul(
                                    ps[:, :],
                                    lhsT=_mm(wt[d][:, e4 * 128:(e4 + 1) * 128],
                                             c_in),
                                    rhs=_mm(xt[d][:, :], c_in),
                                    start=(d == 0), stop=(d == ND - 1))
                            nc.vector.tensor_scalar_add(
                                out=qkT[e][:, l0:l0 + 512], in0=ps[:, :],
                                scalar1=bqk_sb[:, e:e + 1])
                    for ls in range(4):
                        lt = lb * 4 + ls
                        ps = pmm.tile([128, 512], F32, tag="ps", name=f"v{lt}")
                        for d in range(ND):
                            nc.tensor.matmul(
                                ps[:, :],
                                lhsT=_mm(xt[d][:, ls * 128:(ls + 1) * 128], c_in),
                                rhs=_mm(wv_sb[d][:, :], c_in),
                                start=(d == 0), stop=False)
                        nc.tensor.matmul(
                            ps[:, :], lhsT=_mm(ones_col[:, :], c_in),
                            rhs=_mm(bv_sb[:, :], c_in), start=False, stop=True)
                        dst = vaug[lt].rearrange("p (h c) -> p h c", c=65)[:, :, 0:64]
                        src = ps.rearrange("p (h c) -> p h c", c=64)
                        nc.vector.tensor_copy(dst, src)

                    # ---- causal attention for q-block qj = lb --------------
                    # heads run in (even, odd) pairs: even heads sit at SBUF
                    # partitions 0-63, odd at 64-127, so their K=64 score
                    # matmuls land in different PE row groups and overlap.
                    # Two ki-tiles share one 2-bank psum so exp batches to
                    # [128, 1024] (halves the ACT per-op overhead).
                    if phases < 2:
                        continue
                    qj = lb
                    blocks = sched[qj]
                    nblk = len(blocks)
                    pairs = [blocks[i:i + 2] for i in range(0, nblk, 2)]
                    for hp in range(HLOC // 2):
                        t = hp
                        po = {}
                        for h in (2 * hp, 2 * hp + 1):
                            po[h] = pov.tile([65, 512], F32, tag="po",
                                             name=f"o{qj}_{h}")
                        for pi, pair in enumerate(pairs):
                            pw = 512 * len(pair)
                            ps, pt = {}, {}
                            for h in (2 * hp, 2 * hp + 1):
                                base = (h % 2) * 64
                                qslice = qkT[t][base:base + 64, l0:l0 + 512]
                                ps[h] = psc.tile([128, 1024], F32, tag="ps",
                                                 name=f"s{qj}_{h}_{pi}")
                                for j, (ki, pat) in enumerate(pair):
                                    kslice = qkT[4 + t][base:base + 64,
                                                        ki * 128:(ki + 1) * 128]
                                    nc.tensor.matmul(
                                        ps[h][:, j * 512:(j + 1) * 512],
                                        lhsT=_mm(kslice, c_qk),
                                        rhs=_mm(qslice, c_qk),
                                        start=True, stop=True)
                            for h in (2 * hp, 2 * hp + 1):
                                pt[h] = ptp.tile([128, 1024], dt_pv, tag="pt",
                                                 name=f"p{qj}_{h}_{pi}")
                                nc.scalar.activation(
                                    out=pt[h][:, :pw], in_=ps[h][:, :pw],
                                    func=mybir.ActivationFunctionType.Exp)
                                for j, (ki, pat) in enumerate(pair):
                                    if pat is not None:
                                        nc.vector.tensor_mul(
                                            pt[h][:, j * 512:(j + 1) * 512],
                                            pt[h][:, j * 512:(j + 1) * 512],
                                            mask_sb[pat][:, :])
                            for h in (2 * hp, 2 * hp + 1):
                                for j, (ki, pat) in enumerate(pair):
                                    nc.tensor.matmul(
                                        po[h][:, :],
                                        lhsT=_mm(vaug[ki][:, h * 65:(h + 1) * 65],
                                                 c_pv),
                                        rhs=_mm(pt[h][:, j * 512:(j + 1) * 512],
                                                c_pv),
                                        start=(pi == 0 and j == 0),
                                        stop=(pi == len(pairs) - 1
                                              and j == len(pair) - 1))
                        for h in (2 * hp, 2 * hp + 1):
                            base = (h % 2) * 64
                            # single 65-row copy so the po psum slot frees
                            # after one op; recip then reads the SBUF copy
                            osb = osp.tile([65, 512], F32, tag="osb",
                                           name=f"ob{qj}_{h}")
                            nc.vector.tensor_copy(osb[:, :], po[h][0:65, :])
                            rsb = rsp.tile([1, 512], F32, tag="rsb",
                                           name=f"r{qj}_{h}")
                            nc.vector.reciprocal(rsb[:, :], osb[64:65, :])
                            rep = repp.tile([64, 512], F32, tag="rep",
                                            name=f"rp{qj}_{h}")
                            nc.gpsimd.partition_broadcast(
                                rep[:, :], rsb[:, :], channels=64)
                            nc.vector.tensor_mul(
                                yT[t][base:base + 64, l0:l0 + 512],
                                osb[0:64, :], rep[:, :])

                    # ---- output projection for this q-block ----------------
                    if phases < 3:
                        continue
                    for e2 in range(2):
                        for ls in range(4):
                            lr = l0 + ls * 128
                            ps = pmm.tile([128, 512], F32, tag="ps",
                                          name=f"pj{qj}_{e2}_{ls}")
                            for f in range(4):
                                nc.tensor.matmul(
                                    ps[:, :],
                                    lhsT=_mm(yT[f][:, lr:lr + 128], c_proj),
                                    rhs=_mm(projT_sb[f][:, e2 * 512:(e2 + 1) * 512],
                                            c_proj),
                                    start=(f == 0), stop=(f == 3))
                            ob = outp.tile([128, 512], F32, tag="ob",
                                           name=f"yo{qj}_{e2}_{ls}")
                            nc.vector.tensor_copy(ob[:, :], ps[:, :])
                            nc.sync.dma_start(
                                y[lr:lr + 128, e2 * 512:(e2 + 1) * 512], ob[:, :])
    return nc


def make_core_inputs(x, attn_mask, qkv_w, qkv_b, proj_w, patterns, cfg,
                     Lc=L):
    """Host-side shard prep: per-core input dicts for cores 0..7."""
    c_in, c_qk, c_pv, c_proj, c_rep = cfg
    np_in, np_pv, np_proj = _np_st(c_in), _np_st(c_pv), _np_st(c_proj)

    n_pat = max(len(patterns), 1)
    mask_arr = np.zeros((n_pat, 128, 512), np.float32)
    for i, p in enumerate(patterns):
        mask_arr[i] = p
    mask_arr = mask_arr.astype(np_pv)

    in_maps = []
    shared = {}
    for c in range(NCORES):
        b, hg = c // 2, c % 2
        if b not in shared:
            shared[b] = np.ascontiguousarray(
                np.asarray(x[b], np.float32).T).astype(np_in)
        key = ("w", hg)
        if key not in shared:
            rq = qkv_w[hg * 512:hg * 512 + 512, :] * SCALE
            rk = qkv_w[1024 + hg * 512:1024 + hg * 512 + 512, :]
            rv = qkv_w[2048 + hg * 512:2048 + hg * 512 + 512, :]
            w_qk_h = np.ascontiguousarray(
                np.concatenate([rq, rk], 0).T).astype(np_in)
            w_v_h = np.ascontiguousarray(rv.T).astype(np_in)
            bq = qkv_b[hg * 512:hg * 512 + 512] * SCALE
            bk = qkv_b[1024 + hg * 512:1024 + hg * 512 + 512]
            bqk_h = np.ascontiguousarray(
                np.concatenate([bq, bk]).reshape(8, 128).T).astype(np.float32)
            bv_h = np.ascontiguousarray(
                qkv_b[2048 + hg * 512:2048 + hg * 512 + 512].reshape(1, 512)
            ).astype(np_in)
            projT_h = np.ascontiguousarray(
                proj_w[:, hg * 512:hg * 512 + 512].T).astype(np_proj)
            shared[key] = (w_qk_h, w_v_h, bqk_h, bv_h, projT_h)
        w_qk_h, w_v_h, bqk_h, bv_h, projT_h = shared[("w", hg)]
        in_maps.append({
            "xT": shared[b],
            "w_qk": w_qk_h,
            "w_v": w_v_h,
            "bqk": bqk_h,
            "bv": bv_h,
            "masks": mask_arr,
            "projT": projT_h,
        })
    return in_maps


_NC_CACHE = {}
LAST_RESULTS = None


def kernel(**inputs):
    x = np.asarray(inputs["x"], np.float32)
    attn_mask = np.asarray(inputs["attn_mask"])
    qkv_w = np.asarray(inputs["qkv_w"], np.float32)
    qkv_b = np.asarray(inputs["qkv_b"], np.float32)
    proj_w = np.asarray(inputs["proj_w"], np.float32)
    proj_b = np.asarray(inputs["proj_b"], np.float32)

    cfg = DEFAULT_CFG
    sched, patterns = schedule_from_mask(attn_mask, L)

    key = (L, tuple(tuple(r) for r in sched), len(patterns), cfg)
    if key not in _NC_CACHE:
        nc = build_nc(L, sched, len(patterns), cfg)
        if not nc.is_finalized():
            nc.finalize()  # bacc regalloc etc.; bass2jax serializes as-is
        _NC_CACHE[key] = nc
    nc = _NC_CACHE[key]

    in_maps = make_core_inputs(x, attn_mask, qkv_w, qkv_b, proj_w, patterns,
                               cfg)
    res = run_bass_kernel_spmd(nc, in_maps, list(range(NCORES)))
    global LAST_RESULTS
    LAST_RESULTS = res

    out = np.empty((B, L, DIM), np.float32)
    for b in range(B):
        out[b] = (res.results[2 * b]["y"] + res.results[2 * b + 1]["y"]
                  + proj_b)
    return out



# revision 44
# speedup vs baseline: 1.5195x; 1.5195x over previous
"""Causal self-attention (B=4, L=2048, D=1024, H=16) on 8 Trainium2 NeuronCores.

Sharding: core c handles batch b = c//2 and head-group hg = c%2 (8 of 16 heads).
Each core computes its local QKV projection, causal flash-style attention for
its 8 heads, and a partial output projection against its 512 columns of
proj_w. The host sums the two partial outputs per batch and adds proj_b.

Device layouts (per core):
  xT    [1024, L]   x[b].T              (contraction dim d on partitions)
  w_qk  [1024,1024] qkv_w local q+k rows, transposed; q part pre-scaled by
                    HEAD_DIM**-0.5 (folded into weights+bias)
  w_v   [1024, 512] qkv_w local v rows, transposed
  qkT   [1024, L]   (q;k) features on partitions, tokens on free dim
  vaug  [L, 8*65]   v in token-major layout, one extra ones-column per head
                    (the ones column makes the PV matmul also produce the
                    softmax denominator as psum row 64)
  yT    [512, L]    normalized attention output, features on partitions
  projT [512, 1024] proj_w local columns, transposed

Softmax skips the max-subtraction (scores are O(+-10) for this problem's
scale, far from fp32 overflow) so P = exp(S); masking is a multiplicative
0/1 mask applied only on blocks that the host classifies as partial. Blocks
that are fully masked are skipped; fully-kept blocks skip the mask multiply.
"""

import os
import sys

import numpy as np

for _p in ("/opt/trn_rl_repo", "/root/.axon_site/_ro/trn_rl_repo"):
    if os.path.isdir(_p) and _p not in sys.path:
        sys.path.append(_p)

import ml_dtypes  # noqa: E402
import concourse.bass as bass  # noqa: E402
import concourse.tile as tile  # noqa: E402
from concourse import bacc, mybir  # noqa: E402
from concourse.bass_utils import run_bass_kernel_spmd  # noqa: E402

DIM = 1024
NUM_HEADS = 16
HEAD_DIM = 64
SCALE = HEAD_DIM**-0.5
B = 4
L = 2048
NCORES = 8
HLOC = 8  # heads per core

F32 = mybir.dt.float32
BF16 = mybir.dt.bfloat16
F32R = mybir.dt.float32r

# dtype config per matmul stage: "f32" | "bf16" | "f32r"
# f32r stores fp32 bits and bitcasts the matmul operands to float32r
# (full-rate PE streaming, ~tf32 multiply precision).
DEFAULT_CFG = (
    os.environ.get("MM_IN", "bf16"),    # x / w_qk / w_v (QKV projection)
    os.environ.get("MM_QK", "bf16"),    # q/k storage + scores matmul
    os.environ.get("MM_PV", "bf16"),    # P / v_aug (attention-weighted sum)
    os.environ.get("MM_PROJ", "bf16"),  # yT / projT (output projection)
    os.environ.get("MM_REP", "f32"),    # (unused since gpsimd broadcast)
)


def _st(c):
    """Storage dtype for a config string."""
    return {"bf16": BF16, "f32r": F32R, "f32": F32}[c]


def _np_st(c):
    return ml_dtypes.bfloat16 if c == "bf16" else np.float32


def _mm(ap, c):
    """Matmul-operand view of an AP for a config string."""
    return ap


def schedule_from_mask(am, Lc):
    """Classify [128k x 512q] score blocks from attn_mask[q, k].

    Returns (sched, patterns): sched[qj] = list of (ki, pat_idx|None) blocks
    to compute; patterns = list of [128, 512] float32 0/1 arrays (k on
    partitions, q on free dim) for partially-masked blocks.
    """
    am = np.asarray(am) != 0
    sched, patterns, pat_ids = [], [], {}
    for qj in range(Lc // 512):
        row = []
        for ki in range(Lc // 128):
            blk = am[qj * 512:(qj + 1) * 512, ki * 128:(ki + 1) * 128]  # [q,k]
            if not blk.any():
                continue
            if blk.all():
                row.append((ki, None))
                continue
            pat = np.ascontiguousarray(blk.T).astype(np.float32)  # [k,q]
            key = pat.tobytes()
            if key not in pat_ids:
                pat_ids[key] = len(patterns)
                patterns.append(pat)
            row.append((ki, pat_ids[key]))
        sched.append(row)
    return sched, patterns


def pair_patterns(sched, patterns):
    """Merge per-block masks into per-[128,1024] ki-pair masks.

    Returns (pair_pat, pats2): pair_pat[(qj, pi)] = index into pats2 for
    pairs that need masking; pats2 = list of [128, 1024] float32 arrays.
    Halves the DVE mask-multiply count vs per-block masks.
    """
    pair_pat, pats2, ids = {}, [], {}
    ones = np.ones((128, 512), np.float32)
    for qj, row in enumerate(sched):
        pairs = [row[i:i + 2] for i in range(0, len(row), 2)]
        for pi, pair in enumerate(pairs):
            if all(p is None for (_, p) in pair):
                continue
            parts = [ones if p is None else patterns[p] for (_, p) in pair]
            if len(parts) == 1:
                parts.append(ones)
            m = np.concatenate(parts, axis=1)  # [128, 1024]
            key = m.tobytes()
            if key not in ids:
                ids[key] = len(pats2)
                pats2.append(m)
            pair_pat[(qj, pi)] = ids[key]
    return pair_pat, pats2


ABL = set(filter(None, os.environ.get("ABL", "").split(",")))


def build_nc(Lc, sched, pair_pat, n_pat, cfg, nrep=1, phases=3):
    """Emit the per-core Bass/Tile program. Same program runs on all cores.

    nrep > 1 wraps the whole body in an on-device For_i loop — used only
    by the dev timing harness to amortize host/dispatch overhead.
    phases: 1 = QKV only, 2 = +attention, 3 = full (dev decomposition).
    """
    c_in, c_qk, c_pv, c_proj, c_rep = cfg
    dt_in, dt_qk, dt_pv, dt_proj, dt_rep = (
        _st(c_in), _st(c_qk), _st(c_pv), _st(c_proj), _st(c_rep))

    NLB = Lc // 512   # l-blocks (also q-blocks)
    NKT = Lc // 128   # k-tiles
    ND = DIM // 128   # contraction tiles for QKV
    assert n_pat <= 16, f"too many distinct mask patterns ({n_pat})"

    nc = bacc.Bacc("TRN2", target_bir_lowering=False, debug=False)

    dt_c = F32 if dt_in == F32R else dt_in
    dt_rc = F32 if dt_rep == F32R else dt_rep

    xT = nc.dram_tensor("xT", [DIM, Lc], dt_in, kind="ExternalInput")
    # packed input weights: cols 0-1023 = q|k rows (transposed, q pre-scaled),
    # cols 1024-1535 = v rows. One [128, 1536] tile per d gives 3KB DMA lines.
    w_in = nc.dram_tensor("w_in", [DIM, 1536], dt_in, kind="ExternalInput")
    bqk = nc.dram_tensor("bqk", [128, 8], F32, kind="ExternalInput")
    masks = nc.dram_tensor(
        "masks", [max(n_pat, 1), 128, 1024], dt_pv, kind="ExternalInput")
    projT = nc.dram_tensor("projT", [512, 1024], dt_proj, kind="ExternalInput")
    y = nc.dram_tensor("y", [Lc, 1024], F32, kind="ExternalOutput")

    with tile.TileContext(nc) as tc:
        import contextlib
        with contextlib.ExitStack() as ctx:
            sing = ctx.enter_context(tc.tile_pool(name="sing", bufs=1))

            # persistent buffers
            qkT = [sing.tile([128, Lc], dt_qk, tag=f"qkT{t}", name=f"qkT{t}")
                   for t in range(8)]
            vaug = [sing.tile([128, HLOC * 65], dt_pv, tag=f"vaug{t}",
                              name=f"vaug{t}") for t in range(NKT)]
            yT = [sing.tile([128, Lc], dt_proj, tag=f"yT{t}", name=f"yT{t}")
                  for t in range(4)]
            projT_sb = [sing.tile([128, 1024], dt_proj, tag=f"pw{t}",
                                  name=f"pw{t}") for t in range(4)]
            win_sb = [sing.tile([128, 1536], dt_in, tag=f"win{t}",
                                name=f"win{t}") for t in range(ND)]
            x_sb = [sing.tile([128, Lc], dt_in, tag=f"xsb{t}",
                              name=f"xsb{t}") for t in range(ND)]
            # constant/bias tiles stay plain f32 — memset can't emit f32r,
            # and the rank-1 matmuls that read them don't need f32r speed
            bqk_sb = sing.tile([128, 8], F32, tag="bqk_sb", name="bqk_sb")
            mask_sb = [sing.tile([128, 1024], dt_pv, tag=f"msk{p}",
                                 name=f"msk{p}") for p in range(n_pat)]

            for t in range(NKT):
                va = vaug[t].rearrange("p (h c) -> p h c", c=65)
                nc.vector.memset(va[:, :, 64:65], 1.0)

            # streaming pools
            ptp = ctx.enter_context(tc.tile_pool(name="ptp", bufs=6))
            osp = ctx.enter_context(tc.tile_pool(name="osp", bufs=3))
            rsp = ctx.enter_context(tc.tile_pool(name="rsp", bufs=4))
            repp = ctx.enter_context(tc.tile_pool(name="repp", bufs=3))
            outp = ctx.enter_context(tc.tile_pool(name="outp", bufs=3))
            # psum: 2 score slots [128,1024] (4 banks) + 2 qkv/proj slots
            # [128,512] (2 banks) + 2 po slots (2 banks) = 8 banks.
            pmm = ctx.enter_context(tc.tile_pool(name="pmm", bufs=2,
                                                 space="PSUM"))
            psc = pmm
            pq = ctx.enter_context(tc.tile_pool(
                name="pq", bufs=3 if "pq3" in ABL else 2, space="PSUM"))
            pov = ctx.enter_context(tc.tile_pool(
                name="pov", bufs=1 if "pq3" in ABL else 2, space="PSUM"))

            def qkv_groups(lb):
                """One closure per psum group of the QKV projection."""
                l0 = lb * 512
                gs = []
                for eh in range(2):
                    for e4 in range(4):
                        def g(eh=eh, e4=e4):
                            e = eh * 4 + e4
                            c0 = eh * 512 + e4 * 128
                            ps = pq.tile([128, 512], F32, tag="pq",
                                         name=f"q{lb}_{e}")
                            for d in range(ND):
                                nc.tensor.matmul(
                                    ps[:, :],
                                    lhsT=_mm(win_sb[d][:, c0:c0 + 128], c_in),
                                    rhs=_mm(x_sb[d][:, l0:l0 + 512], c_in),
                                    start=(d == 0), stop=(d == ND - 1))
                            # bias-add on ACT (idle during QKV): frees DVE
                            # and turns the pq psum slot over without the
                            # DVE queue in the loop
                            nc.scalar.activation(
                                out=qkT[e][:, l0:l0 + 512], in_=ps[:, :],
                                func=mybir.ActivationFunctionType.Identity,
                                bias=bqk_sb[:, e:e + 1])
                        gs.append(g)
                for ls in range(4):
                    def g(ls=ls):
                        # no v-bias matmul: bv's contribution to the output
                        # is projT^T @ bv — a constant vector the host folds
                        # into proj_b (softmax denominators cancel it out of
                        # the normalize exactly as for unbiased v).
                        lt = lb * 4 + ls
                        lr = l0 + ls * 128
                        ps = pq.tile([128, 512], F32, tag="pq", name=f"v{lt}")
                        for d in range(ND):
                            nc.tensor.matmul(
                                ps[:, :],
                                lhsT=_mm(x_sb[d][:, lr:lr + 128], c_in),
                                rhs=_mm(win_sb[d][:, 1024:1536], c_in),
                                start=(d == 0), stop=(d == ND - 1))
                        dst = vaug[lt].rearrange(
                            "p (h c) -> p h c", c=65)[:, :, 0:64]
                        src = ps.rearrange("p (h c) -> p h c", c=64)
                        if "actv" in ABL:
                            nc.scalar.activation(
                                out=dst, in_=src,
                                func=mybir.ActivationFunctionType.Copy)
                        else:
                            nc.vector.tensor_copy(dst, src)
                    gs.append(g)
                return gs

            def proj_groups(qj):
                """One closure per psum group of the output projection."""
                gs = []
                for e2 in range(2):
                    for ls in range(4):
                        def g(e2=e2, ls=ls):
                            lr = qj * 512 + ls * 128
                            ps = pq.tile([128, 512], F32, tag="pq",
                                         name=f"pj{qj}_{e2}_{ls}")
                            for f in range(4):
                                nc.tensor.matmul(
                                    ps[:, :],
                                    lhsT=_mm(yT[f][:, lr:lr + 128], c_proj),
                                    rhs=_mm(projT_sb[f][:, e2 * 512:
                                                        (e2 + 1) * 512],
                                            c_proj),
                                    start=(f == 0), stop=(f == 3))
                            ob = outp.tile([128, 512], F32, tag="ob",
                                           name=f"yo{qj}_{e2}_{ls}")
                            if "actob" in ABL:
                                nc.scalar.activation(
                                    out=ob[:, :], in_=ps[:, :],
                                    func=mybir.ActivationFunctionType.Copy)
                            else:
                                nc.vector.tensor_copy(ob[:, :], ps[:, :])
                            if "yout" not in ABL:
                                nc.sync.dma_start(
                                    y[lr:lr + 128, e2 * 512:(e2 + 1) * 512],
                                    ob[:, :])
                        gs.append(g)
                return gs

            def attention(qj, fillers):
                """Causal attention for q-block qj, one head at a time.

                PV is lagged one ki-pair behind the scores so the exp (ACT)
                latency never lands on the PE critical path, and `fillers`
                (next block's QKV groups + previous block's proj groups —
                pure PE work with no ACT dependency) are interleaved evenly
                between pair-iterations to fill the PE's exp-wait gaps.
                """
                l0 = qj * 512
                blocks = sched[qj]
                pairs = [blocks[i:i + 2] for i in range(0, len(blocks), 2)]
                total_iters = HLOC * len(pairs)
                it = fi = 0
                pending_mul = []  # yT-muls lagged one head (DVE HOL avoid)

                def flush_muls():
                    while pending_mul:
                        tm, bm, osbm, repm = pending_mul.pop(0)
                        nc.vector.tensor_mul(
                            yT[tm][bm:bm + 64, l0:l0 + 512],
                            osbm[0:64, :], repm[:, :])

                for h in range(HLOC):
                    t = h // 2
                    base = (h % 2) * 64
                    qslice = qkT[t][base:base + 64, l0:l0 + 512]
                    po = pov.tile([65, 512], F32, tag="po", name=f"o{qj}_{h}")
                    pend = None  # (pt, pair, pi) awaiting PV emission

                    def emit_pv(pt, pair, pi, po=None, h=None):
                        # one [128, pw] mask-mul per pair, lagged to PV time:
                        # its exp completed a full pair-iteration ago, so the
                        # DVE never head-of-line blocks waiting on ACT.
                        pp = pair_pat.get((qj, pi))
                        if pp is not None and "mask" not in ABL:
                            pw = 512 * len(pair)
                            nc.vector.tensor_mul(
                                pt[:, :pw], pt[:, :pw], mask_sb[pp][:, :pw])
                        for j, (ki, pat) in enumerate(pair):
                            nc.tensor.matmul(
                                po[:, :],
                                lhsT=_mm(vaug[ki][:, h * 65:(h + 1) * 65],
                                         c_pv),
                                rhs=_mm(pt[:, j * 512:(j + 1) * 512], c_pv),
                                start=(pi == 0 and j == 0),
                                stop=(pi == len(pairs) - 1
                                      and j == len(pair) - 1))

                    for pi, pair in enumerate(pairs):
                        pw = 512 * len(pair)
                        ps = psc.tile([128, 1024], F32, tag="ps",
                                      name=f"s{qj}_{h}_{pi}")
                        for j, (ki, pat) in enumerate(pair):
                            kslice = qkT[4 + t][base:base + 64,
                                                ki * 128:(ki + 1) * 128]
                            nc.tensor.matmul(
                                ps[:, j * 512:(j + 1) * 512],
                                lhsT=_mm(kslice, c_qk),
                                rhs=_mm(qslice, c_qk),
                                start=True, stop=True)
                        pt = ptp.tile([128, 1024], dt_pv, tag="pt",
                                      name=f"p{qj}_{h}_{pi}")
                        nc.scalar.activation(
                            out=pt[:, :pw], in_=ps[:, :pw],
                            func=mybir.ActivationFunctionType.Exp)
                        if pend is not None:
                            emit_pv(*pend, po=po, h=h)
                        pend = (pt, pair, pi)
                        it += 1
                        while fi * total_iters < len(fillers) * it:
                            fillers[fi]()
                            fi += 1
                    emit_pv(*pend, po=po, h=h)

                    # normalize: off the PE path (proj is deferred)
                    if "norm" in ABL:
                        nc.vector.tensor_copy(
                            yT[t][base:base + 64, l0:l0 + 512], po[0:64, :])
                        continue
                    osb = osp.tile([65, 512], F32, tag="osb",
                                   name=f"ob{qj}_{h}")
                    nc.vector.tensor_copy(osb[:, :], po[0:65, :])
                    # denom to a partition-0 tile: reciprocal_approx_fast's
                    # ucode misreads inputs at nonzero partition offsets.
                    dn = rsp.tile([1, 512], F32, tag="dn", name=f"d{qj}_{h}")
                    nc.vector.tensor_copy(dn[:, :], po[64:65, :])
                    rsb = rsp.tile([1, 512], F32, tag="rsb", name=f"r{qj}_{h}")
                    nc.vector.reciprocal_approx_fast(rsb[:, :], dn[:, :])
                    rep = repp.tile([64, 512], F32, tag="rep",
                                    name=f"rp{qj}_{h}")
                    nc.gpsimd.partition_broadcast(
                        rep[:, :], rsb[:, :], channels=64)
                    # yT-mul lagged one head: by the time it issues, its
                    # broadcast is long done, so the DVE queue never head-of-
                    # line blocks waiting on the Pool engine.
                    flush_muls()
                    pending_mul.append((t, base, osb, rep))
                flush_muls()
                while fi < len(fillers):
                    fillers[fi]()
                    fi += 1

            rep_ctx = (tc.For_i(0, nrep, 1) if nrep > 1
                       else contextlib.nullcontext())
            with rep_ctx:
                # DMA issue order = first-use order; w/x tiles interleaved
                # per-d so the d=0 accumulation steps of QKV(0) can start
                # as soon as the first pair lands.
                for d in range(ND):
                    nc.sync.dma_start(win_sb[d][:, :],
                                      w_in[d * 128:(d + 1) * 128, :])
                    nc.sync.dma_start(x_sb[d][:, :],
                                      xT[d * 128:(d + 1) * 128, :])
                nc.sync.dma_start(bqk_sb[:, :], bqk[:, :])
                for p in range(n_pat):
                    nc.sync.dma_start(mask_sb[p][:, :], masks[p, :, :])

                if phases < 2:
                    for lb in range(NLB):
                        for g in qkv_groups(lb):
                            g()
                else:
                    # prologue: QKV(0) sequential; QKV(lb+1) and proj(lb-1)
                    # then ride inside attention(lb) as PE filler.
                    for g in qkv_groups(0):
                        g()
                    for lb in range(NLB):
                        fillers = []
                        if lb + 1 < NLB:
                            fillers += qkv_groups(lb + 1)
                        if phases >= 3 and lb == 0 and nrep > 1:
                            # steady-state pipelining: the previous
                            # iteration's last projection rides here instead
                            # of draining serially at the iteration boundary
                            fillers += proj_groups(NLB - 1)
                        if phases >= 3 and lb == 1:
                            for t in range(4):
                                nc.sync.dma_start(
                                    projT_sb[t][:, :],
                                    projT[t * 128:(t + 1) * 128, :])
                        if phases >= 3 and lb == NLB - 1:
                            # all other deferred projections ride in the last
                            # (longest, ACT-bound) attention block's slack
                            for qj in range(NLB - 1):
                                fillers += proj_groups(qj)
                        attention(lb, fillers)
                    if phases >= 3 and nrep == 1:
                        for g in proj_groups(NLB - 1):
                            g()
    return nc


def make_core_inputs(x, attn_mask, qkv_w, qkv_b, proj_w, patterns, cfg,
                     Lc=L):
    """Host-side shard prep: per-core input dicts for cores 0..7."""
    c_in, c_qk, c_pv, c_proj, c_rep = cfg
    np_in, np_pv, np_proj = _np_st(c_in), _np_st(c_pv), _np_st(c_proj)

    n_pat = max(len(patterns), 1)
    mask_arr = np.zeros((n_pat, 128, 1024), np.float32)
    for i, p in enumerate(patterns):
        mask_arr[i] = p
    mask_arr = mask_arr.astype(np_pv)

    in_maps = []
    shared = {}
    for c in range(NCORES):
        b, hg = c // 2, c % 2
        if b not in shared:
            shared[b] = np.ascontiguousarray(
                np.asarray(x[b], np.float32).T).astype(np_in)
        key = ("w", hg)
        if key not in shared:
            rq = qkv_w[hg * 512:hg * 512 + 512, :] * SCALE
            rk = qkv_w[1024 + hg * 512:1024 + hg * 512 + 512, :]
            rv = qkv_w[2048 + hg * 512:2048 + hg * 512 + 512, :]
            w_in_h = np.ascontiguousarray(
                np.concatenate([rq, rk, rv], 0).T).astype(np_in)
            bq = qkv_b[hg * 512:hg * 512 + 512] * SCALE
            bk = qkv_b[1024 + hg * 512:1024 + hg * 512 + 512]
            bqk_h = np.ascontiguousarray(
                np.concatenate([bq, bk]).reshape(8, 128).T).astype(np.float32)
            projT_h = np.ascontiguousarray(
                proj_w[:, hg * 512:hg * 512 + 512].T).astype(np_proj)
            shared[key] = (w_in_h, bqk_h, projT_h)
        w_in_h, bqk_h, projT_h = shared[("w", hg)]
        in_maps.append({
            "xT": shared[b],
            "w_in": w_in_h,
            "bqk": bqk_h,
            "masks": mask_arr,
            "projT": projT_h,
        })
    return in_maps


_NC_CACHE = {}
LAST_RESULTS = None


def kernel(**inputs):
    x = np.asarray(inputs["x"], np.float32)
    attn_mask = np.asarray(inputs["attn_mask"])
    qkv_w = np.asarray(inputs["qkv_w"], np.float32)
    qkv_b = np.asarray(inputs["qkv_b"], np.float32)
    proj_w = np.asarray(inputs["proj_w"], np.float32)
    proj_b = np.asarray(inputs["proj_b"], np.float32)

    cfg = DEFAULT_CFG
    sched, patterns = schedule_from_mask(attn_mask, L)
    pair_pat, pats2 = pair_patterns(sched, patterns)

    key = (L, tuple(tuple(r) for r in sched),
           tuple(sorted(pair_pat.items())), len(pats2), cfg)
    if key not in _NC_CACHE:
        nc = build_nc(L, sched, pair_pat, len(pats2), cfg)
        if not nc.is_finalized():
            nc.finalize()  # bacc regalloc etc.; bass2jax serializes as-is
        _NC_CACHE[key] = nc
    nc = _NC_CACHE[key]

    in_maps = make_core_inputs(x, attn_mask, qkv_w, qkv_b, proj_w, pats2,
                               cfg)
    res = run_bass_kernel_spmd(nc, in_maps, list(range(NCORES)))
    global LAST_RESULTS
    LAST_RESULTS = res

    # v-bias folded host-side: normalized y_head = y_nobias/denom + bv, so
    # the projection picks up the constant proj_w @ bv.
    bias = proj_b + proj_w @ qkv_b[2048:3072]
    out = np.empty((B, L, DIM), np.float32)
    for b in range(B):
        out[b] = (res.results[2 * b]["y"] + res.results[2 * b + 1]["y"]
                  + bias)
    return out

